# revision 16
# baseline (speedup 1.0000x reference)
"""Trainium2 Bass kernel for a transformer decoder block (self-attn + cross-attn + FFN).

Sharding: zero-collective data parallelism over tokens. 8 cores; core c handles
batch b = c//4 and the 4 query blocks {4s + (c%4) : s in 0..3} of 128 tokens
each. Each core redundantly computes full-sequence K/V projections and
everything else only for its own tokens.

v2 datapath (vs v1):
- Q/K/V projections run in fp8e4m3 with DoubleRow perf mode (2 contraction
  planes per pass); their quantization noise averages out through the softmax
  weighted mean. Weights are host-prescaled by 32 and packed flat into the
  exact SBUF layout so every weight DMA is a contiguous copy.
- AV matmul runs fp8-DoubleRow over paired key blocks (probs exp'd straight
  to fp8), with the softmax denominator riding as a ones-column per head.
- Scores, out-proj and FFN run in fp16 (same PE rate as bf16, 8x less
  quantization noise - their noise does NOT average out).
- Causal skip: for query block s only key blocks kg <= 4s+3 are computed
  (query-range restricted per key block); boundary masks are multiplicative
  0/1 tiles applied to the fp8 probabilities on DVE (no PE mask matmuls).
- All transposes go through the DMA XBAR engine (dma_start_transpose, 16-bit),
  not the PE. The encoder stream arrives host-pretransposed/prequantized.
- V stays in SBUF (no DRAM scratch roundtrip).
"""
import sys
import numpy as np
import ml_dtypes

for _p in ('/opt/trn_rl_repo',):
    if _p not in sys.path:
        sys.path.append(_p)

import concourse.bass as bass
import concourse.tile as tile
from concourse import bacc, mybir

P = 128
HD = 64
EPS = 1e-5
WSC = 32.0            # fp8 weight prescale; evictions multiply by 1/WSC

f32 = mybir.dt.float32
f32r = mybir.dt.float32r
f16 = mybir.dt.float16
f8 = mybir.dt.float8e4
AF = mybir.ActivationFunctionType
ALU = mybir.AluOpType
DR = mybir.MatmulPerfMode.DoubleRow


class Cfg:
    def __init__(self, T=2048, D=1024, H=16, FF=4096):
        self.T, self.D, self.H, self.FF = T, D, H, FF
        self.OWN = T // 4          # tokens per core
        self.NQB = self.OWN // P   # own q-blocks (128 each)
        self.DC = D // P           # D chunks
        self.CP = self.DC // 2     # D chunk-pairs (DoubleRow)
        self.FC = FF // P          # FFN chunks
        self.PAIRS = H // 2
        self.KB = T // P           # key blocks (global)
        self.TH = T // 2           # tokens per half
        self.KBH = self.KB // 2    # key blocks per half
        assert self.OWN == 512 and self.NQB == 4 and self.DC == 8


def build_masks8(cfg, j):
    """Multiplicative 0/1 causal masks, one [P, P] tile per key block kg,
    applied to the probabilities of query block s = kg//4 only."""
    m = np.zeros((cfg.KB, P, P), np.float32)
    for kg in range(cfg.KB):
        rel = kg % 4
        if rel < j:
            m[kg] = 1.0
        elif rel == j:
            pidx = np.arange(P)[:, None]   # key within block
            fidx = np.arange(P)[None, :]   # query within block
            m[kg] = (pidx <= fidx).astype(np.float32)
        # rel > j: stays 0
    out = np.ascontiguousarray(m.transpose(1, 0, 2).reshape(P, -1))
    return out.astype(ml_dtypes.float8_e4m3)


def build_nc(cfg, with_gb):
    T, D, H, FF = cfg.T, cfg.D, cfg.H, cfg.FF
    OWN, NQB, DC, CP, FC = cfg.OWN, cfg.NQB, cfg.DC, cfg.CP, cfg.FC
    PAIRS, KB, TH, KBH = cfg.PAIRS, cfg.KB, cfg.TH, cfg.KBH
    ATT_SCALE = float(D) ** -0.5
    IW = 1.0 / WSC

    nc = bacc.Bacc("TRN2", target_bir_lowering=False, debug=False)
    dp = nc.declare_dram_parameter
    x_own = dp("x_own", [OWN, D], f32, isOutput=False)
    xdec = dp("xdec", [T, D], f16, isOutput=False)
    encT8 = dp("encT8", [P, 2 * DC * TH], f8, isOutput=False)
    wq_sa = dp("wq_sa", [PAIRS, P, CP * 2 * P], f8, isOutput=False)
    wk_sa = dp("wk_sa", [PAIRS, P, CP * 2 * P], f8, isOutput=False)
    wv_sa = dp("wv_sa", [P, 2 * CP * 2 * 512], f8, isOutput=False)
    wo_sa = dp("wo_sa", [DC, 64, 2 * PAIRS * P], f16, isOutput=False)
    bo_sa = dp("bo_sa", [P, DC], f32, isOutput=False)
    wq_ca = dp("wq_ca", [PAIRS, P, CP * 2 * P], f8, isOutput=False)
    wk_ca = dp("wk_ca", [PAIRS, P, CP * 2 * P], f8, isOutput=False)
    wv_ca = dp("wv_ca", [P, 2 * CP * 2 * 512], f8, isOutput=False)
    wo_ca = dp("wo_ca", [DC, 64, 2 * PAIRS * P], f16, isOutput=False)
    bo_ca = dp("bo_ca", [P, DC], f32, isOutput=False)
    w1 = dp("w1", [FC // 2, P, 2 * DC * P], f16, isOutput=False)
    b1 = dp("b1", [P, FC], f32, isOutput=False)
    w2 = dp("w2", [4 * DC, P, (FC // 4) * P], f16, isOutput=False)
    b2 = dp("b2", [P, DC], f32, isOutput=False)
    masks = dp("masks", [P, KB * P], f8, isOutput=False)
    gbs = {}
    if with_gb:
        for n in ("g1", "be1", "g2", "be2", "g3", "be3"):
            gbs[n] = dp(n, [P, DC], f32, isOutput=False)
    out = dp("out", [OWN, D], f32, isOutput=True)

    from contextlib import ExitStack
    with tile.TileContext(nc) as tc:
        with ExitStack() as _ctx:
            _ctx.enter_context(nc.allow_low_precision(
                reason="fp8/fp16 datapath, tolerance is absmax-relative 2e-2"))
            _pool = lambda nm, bufs, **kw: _ctx.enter_context(
                tc.tile_pool(name=nm, bufs=bufs, **kw))
            constp = _pool("constp", 1)
            residp = _pool("residp", 4)
            ldp = _pool("ldp", 3)          # f16 x loads [P, D]
            stgp = _pool("stgp", 3)        # LN out / transpose staging f16
            xT8p = _pool("xT8p", 2)        # fp8 x^T mega [P, DC*TH]
            lnq8p = _pool("lnq8p", 2)      # fp8 own-token LN'd x^T [P, DC*OWN]
            lnq16p = _pool("lnq16p", 1)    # f16 LN3'd x^T for FFN
            ktp = _pool("ktp", 3)          # K^T f16 [P, TH]
            qTp = _pool("qTp", 8)          # Q^T f16 [P, OWN]
            v8p = _pool("v8p", 8)          # V fp8 pair tiles [P, 2*(H*65)]
            pb8p = _pool("pb8p", 3)        # probs fp8 [P, 2*OWN]
            avsbp = _pool("avsbp", 8)      # attn accum f16 [65, 2*OWN]
            av16p = _pool("av16p", 8)      # normalized attn f16 [64, 2*OWN]
            wpairp = _pool("wpairp", 3)    # wq/wk pair [P, CP*2*P] fp8
            wvp = _pool("wvp", 2)          # wv [P, 8192] fp8
            wop = _pool("wop", 2)          # wo m-chunk [64, 2048] f16
            w1p = _pool("w1p", 2)          # w1 2f [P, 2048] f16
            w2p = _pool("w2p", 3)          # w2 (qg,m) [P, 1024] f16
            rT16p = _pool("rT16p", 2)      # relu acts f16 [P, 8*OWN]
            y2p = _pool("y2p", 1)          # FFN out f16 [P, DC*OWN]
            tevp = _pool("tevp", 3)        # writeback staging f16 [P, OWN]
            smallp = _pool("smallp", 8)
            recp = _pool("recp", 2)
            bp = _pool("bp", 1)
            scps = _pool("scps", 2, space="PSUM")    # scores [P, 2*OWN]
            avps = _pool("avps", 2, space="PSUM")    # AV accum [65, OWN]
            linps = _pool("linps", 2, space="PSUM")  # GEMM psum [P, OWN]

            # ---------------- constants ----------------
            ones_f = constp.tile([P, 64], f16, tag="ones_f")
            nc.any.memset(ones_f[:], 1.0)
            epst = constp.tile([P, 1], f32, tag="epst")
            nc.any.memset(epst[:], EPS)
            maskt = constp.tile([P, KB * P], f8, tag="maskt")
            nc.sync.dma_start(maskt[:], masks[:])
            r = lambda ap: ap.bitcast(f32r)

            def load_bias(dram, n, nm):
                t = bp.tile([P, n], f32, tag=nm)
                nc.sync.dma_start(t[:], dram[:])
                return t

            gb_tiles = {}
            if with_gb:
                for gk, bk, key in (("g1", "be1", 1), ("g2", "be2", 2),
                                    ("g3", "be3", 3)):
                    gb_tiles[key] = (load_bias(gbs[gk], DC, gk),
                                     load_bias(gbs[bk], DC, bk))
            bo_sa_t = load_bias(bo_sa, DC, "bo_sa_t")
            bo_ca_t = load_bias(bo_ca, DC, "bo_ca_t")
            b1_t = load_bias(b1, FC, "b1_t")
            b2_t = load_bias(b2, DC, "b2_t")

            # ---------------- helpers ----------------
            def emit_ln_stats(xap, width):
                """bn stats over `width` cols -> (rstd, nmrs) [P,1] tiles."""
                nch = (width + 511) // 512
                st6 = smallp.tile([P, nch * 6], f32, tag="st6")
                for i in range(nch):
                    c0, c1 = i * 512, min(width, (i + 1) * 512)
                    nc.vector.bn_stats(st6[:, i * 6:(i + 1) * 6], xap[:, c0:c1])
                mv = smallp.tile([P, 2], f32, tag="mv")
                nc.vector.bn_aggr(mv[:], st6[:].rearrange("p (a b) -> p a b",
                                                          b=6))
                std = smallp.tile([P, 1], f32, tag="std")
                nc.scalar.activation(std[:], mv[:, 1:2], AF.Sqrt, bias=epst[:])
                rstd = smallp.tile([P, 1], f32, tag="rstd")
                nc.vector.reciprocal(rstd[:], std[:])
                nmrs = smallp.tile([P, 1], f32, tag="nmrs")
                nc.vector.tensor_mul(nmrs[:], mv[:, 0:1], rstd[:])
                nc.vector.tensor_scalar_mul(nmrs[:], nmrs[:], -1.0)
                return rstd, nmrs

            def emit_ln_t8(xap, dstv, tslice, gbkey):
                """LN(xap [P, D]) -> f16 -> XBAR transpose -> fp8 into
                dstv [P, DC, *] at column range tslice."""
                rstd, nmrs = emit_ln_stats(xap, D)
                lnb = stgp.tile([P, D], f16, tag="stg")
                nc.gpsimd.tensor_scalar(lnb[:], xap, rstd[:], nmrs[:],
                                        ALU.mult, ALU.add)
                stg = stgp.tile([P, D], f16, tag="stg")
                stgv = stg[:].rearrange("p (d t) -> p d t", d=DC)
                nc.sync.dma_start_transpose(stgv, lnb[:])
                if with_gb and gbkey is not None:
                    gt, bt = gb_tiles[gbkey]
                    for d in range(DC):
                        nc.gpsimd.tensor_scalar(
                            dstv[:, d, tslice], stgv[:, d, :],
                            gt[:, d:d + 1], bt[:, d:d + 1], ALU.mult, ALU.add)
                else:
                    nc.gpsimd.tensor_copy(dstv[:, :, tslice], stgv[:, :, :])

            def stream_dec_half(half, xT8):
                xT8v = xT8[:].rearrange("p (d t) -> p d t", t=TH)
                for tb in range(TH // P):
                    xt = ldp.tile([P, D], f16, tag="ld")
                    r0 = half * TH + tb * P
                    nc.gpsimd.dma_start(xt[:], xdec[r0:r0 + P, :])
                    emit_ln_t8(xt[:], xT8v, slice(tb * P, (tb + 1) * P), 1)

            def dpair(flatv, c, per, t0=None, t1=None):
                """[P, 2, *] DoubleRow view of chunk-pair c from a flat AP
                laid out as (chunk, per)."""
                v = flatv[:, 2 * c * per:(2 * c + 2) * per].rearrange(
                    "p (i t) -> p i t", i=2)
                if t0 is None:
                    return v
                return v[:, :, t0:t1]

            def emit_v(half, xT8, wv8, v8_tiles):
                xT8f = xT8[:]
                for tb in range(TH // P):
                    g, i = tb // 2, tb % 2
                    for nch in range(2):
                        ps = linps.tile([P, OWN], f32, tag="lin")
                        for c in range(CP):
                            rv = wv8[:, (nch * CP + c) * 1024:
                                     (nch * CP + c + 1) * 1024].rearrange(
                                "p (i n) -> p i n", i=2)
                            nc.tensor.matmul(
                                ps[:, :512],
                                dpair(xT8f, c, TH, tb * P, (tb + 1) * P),
                                rv, start=(c == 0), stop=(c == CP - 1),
                                perf_mode=DR)
                        dst = v8_tiles[g][:, i * 1040 + nch * 520:
                                          i * 1040 + (nch + 1) * 520]
                        nc.vector.tensor_scalar_mul(
                            dst.rearrange("p (h c) -> p h c", c=65)[:, :, 0:HD],
                            ps[:, :512].rearrange("p (h c) -> p h c", c=HD),
                            IW)

            def emit_kpair(half, pair, xT8, wkt):
                kt = ktp.tile([P, TH], f16, tag="ktp")
                wv_ = wkt[:]
                for th in range(2):
                    ps = linps.tile([P, OWN], f32, tag="lin")
                    for c in range(CP):
                        nc.tensor.matmul(
                            ps[:, :512], dpair(wv_, c, P),
                            dpair(xT8[:], c, TH, th * 512, (th + 1) * 512),
                            start=(c == 0), stop=(c == CP - 1), perf_mode=DR)
                    nc.vector.tensor_scalar_mul(kt[:, th * 512:(th + 1) * 512],
                                                ps[:, :512], IW)
                return kt

            def emit_qpair(pair, lnq8, wqt):
                qt = qTp.tile([P, OWN], f16, tag="qT")
                ps = linps.tile([P, OWN], f32, tag="lin")
                for c in range(CP):
                    nc.tensor.matmul(
                        ps[:, :OWN], dpair(wqt[:], c, P),
                        dpair(lnq8[:], c, OWN),
                        start=(c == 0), stop=(c == CP - 1), perf_mode=DR)
                nc.vector.tensor_scalar_mul(qt[:], ps[:, :OWN], IW)
                return qt

            def emit_attention(causal, half, pair, qT, kt, v8_tiles, avsb):
                """Scores (f16) -> exp to fp8 probs -> masks -> fp8-DR AV."""
                for hh in range(2):
                    hb = hh * HD
                    avp = avps.tile([65, OWN], f32, tag="av",
                                    name=f"avp{causal}{half}{pair}{hh}")
                    first_off = None
                    for g in range(KBH // 2):
                        kg0 = half * KBH + 2 * g
                        off = (kg0 // 4) * P if causal else 0
                        if first_off is None:
                            first_off = off
                        sc = scps.tile([P, 2 * OWN], f32, tag="sc")
                        for i in range(2):
                            kl = 2 * g + i
                            nc.tensor.matmul(
                                sc[:, i * OWN + off:(i + 1) * OWN],
                                kt[hb:hb + HD, kl * P:(kl + 1) * P],
                                qT[hb:hb + HD, off:OWN],
                                start=True, stop=True)
                        pb = pb8p.tile([P, 2 * OWN], f8, tag="pb")
                        scv = sc[:].rearrange("p (i t) -> p i t", i=2)
                        pbv = pb[:].rearrange("p (i t) -> p i t", i=2)
                        nc.scalar.activation(pbv[:, :, off:OWN],
                                             scv[:, :, off:OWN],
                                             AF.Exp, scale=ATT_SCALE)
                        if causal:
                            for i in range(2):
                                kg = kg0 + i
                                nc.gpsimd.tensor_mul(
                                    pb[:, i * OWN + off:i * OWN + off + P],
                                    pb[:, i * OWN + off:i * OWN + off + P],
                                    maskt[:, kg * P:(kg + 1) * P])
                        h = 2 * pair + hh
                        lv = v8_tiles[g][:].rearrange(
                            "p (i z) -> p i z", i=2)[:, :, h * 65:(h + 1) * 65]
                        nc.tensor.matmul(
                            avp[0:65, off:OWN], lv, pbv[:, :, off:OWN],
                            start=(g == 0), stop=(g == KBH // 2 - 1),
                            perf_mode=DR)
                    cs = hh * OWN
                    if half == 0:
                        nc.vector.tensor_copy(avsb[0:65, cs:cs + OWN],
                                              avp[0:65, :])
                    else:
                        o = first_off
                        nc.vector.tensor_add(avsb[0:65, cs + o:cs + OWN],
                                             avp[0:65, o:OWN],
                                             avsb[0:65, cs + o:cs + OWN])

            def emit_normalize(avsb, av16):
                for hh in range(2):
                    cs = hh * OWN
                    rec = recp.tile([P, OWN], f16, tag="rec")
                    nc.vector.reciprocal(rec[64:65, :],
                                         avsb[64:65, cs:cs + OWN])
                    bc = avps.tile([64, OWN], f32, tag="av")
                    nc.tensor.matmul(bc[:], ones_f[64:65, 0:64],
                                     rec[64:65, :], start=True, stop=True)
                    nc.vector.tensor_mul(av16[0:64, cs:cs + OWN],
                                         avsb[0:64, cs:cs + OWN], bc[:])

            def emit_writeback(ev_ap, res, m):
                """ev_ap [P(m-rows), OWN own tokens] f16 -> transpose -> add
                to residual tiles."""
                tev = tevp.tile([P, OWN], f16, tag="tev")
                nc.sync.dma_start_transpose(
                    tev[:].rearrange("p (s m) -> p s m", s=NQB), ev_ap)
                for s in range(NQB):
                    nc.gpsimd.tensor_add(
                        res[s][:, m * P:(m + 1) * P],
                        tev[:, s * P:(s + 1) * P],
                        res[s][:, m * P:(m + 1) * P])

            def emit_oproj(wo_dram, bo_t, av16_list, res):
                for m in range(DC):
                    wot = wop.tile([64, 2 * PAIRS * P], f16, tag="wop")
                    nc.sync.dma_start(wot[:], wo_dram[m])
                    ps = linps.tile([P, OWN], f32, tag="lin")
                    for n in range(2 * PAIRS):
                        pr, hh = n // 2, n % 2
                        nc.tensor.matmul(
                            ps[:, :OWN], wot[:, n * P:(n + 1) * P],
                            av16_list[pr][0:64, hh * OWN:(hh + 1) * OWN],
                            start=(n == 0), stop=(n == 2 * PAIRS - 1))
                    ev = tevp.tile([P, OWN], f16, tag="tev")
                    nc.scalar.activation(ev[:], ps[:, :OWN], AF.Identity,
                                         bias=bo_t[:, m:m + 1])
                    emit_writeback(ev[:], res, m)

            def emit_lnq8(res, gbkey):
                lnq8 = lnq8p.tile([P, DC * OWN], f8, tag="lnq8")
                lv = lnq8[:].rearrange("p (d t) -> p d t", t=OWN)
                for s in range(NQB):
                    emit_ln_t8(res[s][:], lv, slice(s * P, (s + 1) * P), gbkey)
                return lnq8

            def emit_lnq16(res, gbkey):
                """LN3 -> f16 x^T mega for the FFN (direct XBAR transpose)."""
                lnq = lnq16p.tile([P, DC * OWN], f16, tag="lnq16")
                lv = lnq[:].rearrange("p (d t) -> p d t", t=OWN)
                for s in range(NQB):
                    rstd, nmrs = emit_ln_stats(res[s][:], D)
                    lnb = stgp.tile([P, D], f16, tag="stg")
                    nc.gpsimd.tensor_scalar(lnb[:], res[s][:], rstd[:],
                                            nmrs[:], ALU.mult, ALU.add)
                    if with_gb and gbkey is not None:
                        stg = stgp.tile([P, D], f16, tag="stg")
                        stgv = stg[:].rearrange("p (d t) -> p d t", d=DC)
                        nc.sync.dma_start_transpose(stgv, lnb[:])
                        gt, bt = gb_tiles[gbkey]
                        for d in range(DC):
                            nc.gpsimd.tensor_scalar(
                                lv[:, d, s * P:(s + 1) * P], stgv[:, d, :],
                                gt[:, d:d + 1], bt[:, d:d + 1],
                                ALU.mult, ALU.add)
                    else:
                        nc.sync.dma_start_transpose(
                            lv[:, :, s * P:(s + 1) * P], lnb[:])
                return lnq

            # ================= pipeline =================
            res = []
            for s in range(NQB):
                t = residp.tile([P, D], f32, tag="resid")
                nc.sync.dma_start(t[:], x_own[s * P:(s + 1) * P, :])
                res.append(t)

            # LN1 own tokens -> Q_sa
            lnq1 = emit_lnq8(res, 1)
            qsaT = []
            for pair in range(PAIRS):
                wqt = wpairp.tile([P, CP * 2 * P], f8, tag="wpair",
                                  name=f"wq_sa{pair}")
                nc.sync.dma_start(wqt[:], wq_sa[pair])
                qsaT.append(emit_qpair(pair, lnq1, wqt))

            def attn_side(causal, stream_fn, wv_d, wk_d, qT_list, avsb_list):
                wv8 = wvp.tile([P, 8192], f8, tag="wv")
                nc.sync.dma_start(wv8[:], wv_d[:])
                for half in range(2):
                    xT8 = xT8p.tile([P, DC * TH], f8, tag="xT8")
                    stream_fn(half, xT8)
                    v8_tiles = [v8p.tile([P, 2 * H * 65], f8, tag="v8",
                                         name=f"v8{causal}{half}{g}")
                                for g in range(KBH // 2)]
                    for g in range(KBH // 2):
                        nc.gpsimd.memset(
                            v8_tiles[g][:].rearrange("p (z c) -> p z c",
                                                     c=65)[:, :, 64:65], 1.0)
                    emit_v(half, xT8, wv8[:], v8_tiles)
                    for pair in range(PAIRS):
                        wkt = wpairp.tile([P, CP * 2 * P], f8, tag="wpair",
                                          name=f"wk{causal}{half}{pair}")
                        nc.sync.dma_start(wkt[:], wk_d[pair])
                        kt = emit_kpair(half, pair, xT8, wkt)
                        emit_attention(causal, half, pair, qT_list[pair], kt,
                                       v8_tiles, avsb_list[pair])

            avsb_sa = [avsbp.tile([65, 2 * OWN], f16, tag="avsb",
                                  name=f"avsb_sa{pp}") for pp in range(PAIRS)]
            attn_side(True, stream_dec_half, wv_sa, wk_sa, qsaT, avsb_sa)
            av16_sa = [av16p.tile([64, 2 * OWN], f16, tag="av16",
                                  name=f"av16_sa{pp}") for pp in range(PAIRS)]
            for pair in range(PAIRS):
                emit_normalize(avsb_sa[pair], av16_sa[pair])
            emit_oproj(wo_sa, bo_sa_t, av16_sa, res)

            # LN2 -> Q_ca
            lnq2 = emit_lnq8(res, 2)
            qcaT = []
            for pair in range(PAIRS):
                wqt = wpairp.tile([P, CP * 2 * P], f8, tag="wpair",
                                  name=f"wq_ca{pair}")
                nc.sync.dma_start(wqt[:], wq_ca[pair])
                qcaT.append(emit_qpair(pair, lnq2, wqt))

            def load_enc_half(half, xT8):
                nc.sync.dma_start(
                    xT8[:], encT8[:, half * DC * TH:(half + 1) * DC * TH])

            avsb_ca = [avsbp.tile([65, 2 * OWN], f16, tag="avsb",
                                  name=f"avsb_ca{pp}") for pp in range(PAIRS)]
            attn_side(False, load_enc_half, wv_ca, wk_ca, qcaT, avsb_ca)
            av16_ca = [av16p.tile([64, 2 * OWN], f16, tag="av16",
                                  name=f"av16_ca{pp}") for pp in range(PAIRS)]
            for pair in range(PAIRS):
                emit_normalize(avsb_ca[pair], av16_ca[pair])
            emit_oproj(wo_ca, bo_ca_t, av16_ca, res)

            # LN3 -> FFN (f16, quarter-groups of 8 f-chunks)
            lnq3 = emit_lnq16(res, 3)
            y2 = y2p.tile([P, DC * OWN], f16, tag="y2")
            for qg in range(4):
                rT = rT16p.tile([P, 8 * OWN], f16, tag="rT16")
                for fh in range(4):          # 2 f-chunks per w1 tile
                    w1t = w1p.tile([P, 2 * DC * P], f16, tag="w1t",
                                   name=f"w1t{qg}{fh}")
                    nc.sync.dma_start(w1t[:], w1[qg * 4 + fh])
                    for fl in range(2):
                        f = fh * 2 + fl
                        ps = linps.tile([P, OWN], f32, tag="lin")
                        for c in range(DC):
                            nc.tensor.matmul(
                                ps[:, :OWN],
                                w1t[:, (fl * DC + c) * P:
                                    (fl * DC + c + 1) * P],
                                lnq3[:, c * OWN:(c + 1) * OWN],
                                start=(c == 0), stop=(c == DC - 1))
                        fidx = qg * 8 + f
                        nc.scalar.activation(rT[:, f * OWN:(f + 1) * OWN],
                                             ps[:, :OWN], AF.Relu,
                                             bias=b1_t[:, fidx:fidx + 1])
                for m in range(DC):
                    w2t = w2p.tile([P, 8 * P], f16, tag="w2t",
                                   name=f"w2t{qg}{m}")
                    nc.sync.dma_start(w2t[:], w2[qg * DC + m])
                    ps = linps.tile([P, OWN], f32, tag="lin")
                    for c in range(8):
                        nc.tensor.matmul(
                            ps[:, :OWN], w2t[:, c * P:(c + 1) * P],
                            rT[:, c * OWN:(c + 1) * OWN],
                            start=(c == 0), stop=(c == 7))
                    if qg == 0:
                        nc.scalar.activation(y2[:, m * OWN:(m + 1) * OWN],
                                             ps[:, :OWN], AF.Identity,
                                             bias=b2_t[:, m:m + 1])
                    else:
                        tmp = tevp.tile([P, OWN], f16, tag="tev")
                        nc.scalar.activation(tmp[:], ps[:, :OWN], AF.Identity)
                        nc.gpsimd.tensor_add(y2[:, m * OWN:(m + 1) * OWN],
                                             tmp[:],
                                             y2[:, m * OWN:(m + 1) * OWN])
            for m in range(DC):
                emit_writeback(y2[:, m * OWN:(m + 1) * OWN], res, m)

            for s in range(NQB):
                nc.sync.dma_start(out[s * P:(s + 1) * P, :], res[s][:])

    nc.compile()
    return nc


def own_token_rows(cfg, j):
    return np.concatenate(
        [np.arange(P * (cfg.NQB * s + j), P * (cfg.NQB * s + j) + P)
         for s in range(cfg.NQB)])


def _f8(x):
    return np.ascontiguousarray(x).astype(ml_dtypes.float8_e4m3)


def _f16(x):
    return np.ascontiguousarray(x).astype(np.float16)


def pack_weights(cfg, inputs):
    """Shared (batch-independent) weight packing; computed once."""
    D, H, FF = cfg.D, cfg.H, cfg.FF
    CP, P_, FC, DC, PAIRS = cfg.CP, P, cfg.FC, cfg.DC, cfg.PAIRS
    a = lambda x: np.asarray(x, dtype=np.float32)

    def pack_qk(w):
        w = a(w)
        pr = np.stack([np.concatenate([w[2 * p], w[2 * p + 1]], axis=1)
                       for p in range(PAIRS)])            # [8, D, 128]
        pr = pr.reshape(PAIRS, CP, 2, P_, 128).transpose(0, 3, 1, 2, 4)
        return _f8(pr.reshape(PAIRS, P_, CP * 2 * 128) * WSC)

    def pack_wv(w):
        wall = a(w).transpose(1, 0, 2).reshape(D, H * HD)  # [D, 1024]
        v = wall.reshape(CP, 2, P_, 2, 512).transpose(2, 3, 0, 1, 4)
        return _f8(v.reshape(P_, 2 * CP * 2 * 512) * WSC)

    def pack_wo(w):
        # lhsT per (m, n=(pair,hh)): [64 rows, 128 cols of D]
        w = a(w).reshape(2 * PAIRS, 64, DC, 128).transpose(2, 1, 0, 3)
        return _f16(w.reshape(DC, 64, 2 * PAIRS * 128))

    def pack_w1(w):
        # per 2f-chunk tile: [P, (fl, c, 128out)] with contraction row p,c
        w = a(w).reshape(DC, P_, FC, 128).transpose(2, 1, 0, 3)  # [FC,P,DC,128]
        w = w.reshape(FC // 2, 2, P_, DC * 128).transpose(0, 2, 1, 3)
        return _f16(w.reshape(FC // 2, P_, 2 * DC * 128))

    def pack_w2(w):
        # per (qg, m) tile: [P, (c 8, 128)], contraction row = qg*1024+c*128+p
        w = a(w).reshape(4, 8, P_, DC, 128)            # [qg, c, p, m, mm]
        w = w.transpose(0, 3, 2, 1, 4)                 # [qg, m, p, c, mm]
        return _f16(w.reshape(4 * DC, P_, 8 * 128))

    def pack_bias(b, n):
        return np.ascontiguousarray(a(b).reshape(n, P_).T)

    return {
        "wq_sa": pack_qk(inputs["Wq_sa"]), "wk_sa": pack_qk(inputs["Wk_sa"]),
        "wv_sa": pack_wv(inputs["Wv_sa"]), "wo_sa": pack_wo(inputs["Wo_sa"]),
        "bo_sa": pack_bias(inputs["bo_sa"], DC),
        "wq_ca": pack_qk(inputs["Wq_ca"]), "wk_ca": pack_qk(inputs["Wk_ca"]),
        "wv_ca": pack_wv(inputs["Wv_ca"]), "wo_ca": pack_wo(inputs["Wo_ca"]),
        "bo_ca": pack_bias(inputs["bo_ca"], DC),
        "w1": pack_w1(inputs["W1"]), "b1": pack_bias(inputs["b1"], FC),
        "w2": pack_w2(inputs["W2"]), "b2": pack_bias(inputs["b2"], DC),
    }


def pack_enc(cfg, xe):
    """[T, D] f32 -> [P, 2*DC*TH] fp8, x^T layout (half, d-chunk, token)."""
    v = np.asarray(xe, np.float32).reshape(2, cfg.TH, cfg.DC, P)
    return _f8(v.transpose(3, 0, 2, 1).reshape(P, 2 * cfg.DC * cfg.TH))


def prep_core_inputs(cfg, inputs, core, shared=None, gb_packed=None):
    b, j = core // 4, core % 4
    rows = own_token_rows(cfg, j)
    if shared is None:
        shared = pack_weights(cfg, inputs)
    xd = np.asarray(inputs["decoder_x"], np.float32)[b]
    im = dict(shared)
    im["x_own"] = np.ascontiguousarray(xd[rows])
    im["xdec"] = _f16(xd)
    im["encT8"] = pack_enc(cfg, np.asarray(inputs["encoder_x"],
                                           np.float32)[b])
    im["masks"] = build_masks8(cfg, j)
    if gb_packed:
        im.update(gb_packed)
    return im, rows


def gb_trivial(inputs):
    return all(np.allclose(np.asarray(inputs[g]), 1.0)
               for g in ("g1", "g2", "g3")) and \
           all(np.allclose(np.asarray(inputs[b]), 0.0)
               for b in ("be1", "be2", "be3"))


def pack_gb(cfg, inputs):
    out = {}
    for n in ("g1", "be1", "g2", "be2", "g3", "be3"):
        out[n] = np.ascontiguousarray(
            np.asarray(inputs[n], np.float32).reshape(cfg.DC, P).T)
    return out


def run(inputs, trace=False, **rk):
    """Build + run on 8 cores; returns (full_output, BassKernelResults)."""
    from concourse.bass_utils import run_bass_kernel_spmd

    cfg = Cfg()
    with_gb = not gb_trivial(inputs)
    nc = build_nc(cfg, with_gb)

    shared = pack_weights(cfg, inputs)
    gbp = pack_gb(cfg, inputs) if with_gb else None
    in_maps, rows_all = [], []
    for core in range(8):
        im, rows = prep_core_inputs(cfg, inputs, core, shared, gbp)
        in_maps.append(im)
        rows_all.append(rows)

    res = run_bass_kernel_spmd(nc, in_maps, list(range(8)), trace=trace, **rk)
    full = np.zeros((2, cfg.T, cfg.D), np.float32)
    for core in range(8):
        full[core // 4][rows_all[core]] = res.results[core]["out"]
    return full, res


def kernel(**inputs) -> np.ndarray:
    return run(inputs)[0]


# revision 17
# speedup vs baseline: 1.0736x; 1.0736x over previous
"""Trainium2 Bass kernel for a transformer decoder block (self-attn + cross-attn + FFN).

Sharding: zero-collective data parallelism over tokens. 8 cores; core c handles
batch b = c//4 and the 4 query blocks {4s + (c%4) : s in 0..3} of 128 tokens
each. Each core redundantly computes full-sequence K/V projections (cheaper
than on-chip collectives at this size) and everything else only for its own
tokens. The causal-attention structure is uniform across cores (one SPMD
program); per-core causal masks arrive as input data and are added to scores
with identity matmuls.

Layouts: activations are kept "transposed" ([feature, token]) for matmuls so
weights are always the stationary operand; LayerNorm/softmax-denominator/
residual work happens in [token, feature] layout; PE transposes convert.
Scores are computed as S^T = K^T.T @ Q^T ([key, query]), so the AV matmul
lhsT=[V|ones] both contracts keys and produces the softmax denominator for
free in PSUM row 64.
"""
import sys
import numpy as np
import ml_dtypes

for _p in ('/opt/trn_rl_repo',):
    if _p not in sys.path:
        sys.path.append(_p)

import concourse.bass as bass
import concourse.tile as tile
from concourse import bacc, mybir
from concourse.masks import make_identity

P = 128
HD = 64
EPS = 1e-5
NEG = -1e9

f32 = mybir.dt.float32
f32r = mybir.dt.float32r
bf16 = mybir.dt.bfloat16
AF = mybir.ActivationFunctionType


class Cfg:
    def __init__(self, T=2048, D=1024, H=16, FF=4096):
        self.T, self.D, self.H, self.FF = T, D, H, FF
        self.OWN = T // 4          # tokens per core
        self.NQB = self.OWN // P   # own q-blocks (128 each)
        self.DC = D // P           # D chunks
        self.FC = FF // P          # FFN chunks
        self.PAIRS = H // 2
        self.KB = T // P           # key blocks (global)
        self.TH = T // 2           # tokens per half
        self.KBH = self.KB // 2    # key blocks per half
        assert self.OWN <= 512 and self.T % 256 == 0 and D % P == 0
        assert H % 2 == 0 and self.FC % 2 == 0
        # active mask positions: (s, k) that are not always-visible for
        # every core j: k >= NQB*s  (g_s = NQB*s + j >= NQB*s)
        self.mask_pos = [(s, k) for s in range(self.NQB)
                         for k in range(self.NQB * s, self.KB)]
        self.mask_idx = {sk: i for i, sk in enumerate(self.mask_pos)}


def build_masks(cfg, j):
    """Additive causal mask tiles for core j: [P, n_active*P] f32."""
    m = np.zeros((len(cfg.mask_pos), P, P), np.float32)
    for i, (s, k) in enumerate(cfg.mask_pos):
        g = cfg.NQB * s + j
        if k < g:
            continue                      # fully visible: additive zero
        elif k == g:
            pidx = np.arange(P)[:, None]  # key within block
            fidx = np.arange(P)[None, :]  # query within block
            m[i] = np.where(pidx <= fidx, 0.0, NEG)
        else:
            m[i] = NEG
    return np.ascontiguousarray(m.transpose(1, 0, 2).reshape(P, -1))


def build_nc(cfg, with_gb):
    T, D, H, FF = cfg.T, cfg.D, cfg.H, cfg.FF
    OWN, NQB, DC, FC = cfg.OWN, cfg.NQB, cfg.DC, cfg.FC
    PAIRS, KB, TH, KBH = cfg.PAIRS, cfg.KB, cfg.TH, cfg.KBH
    NACT = len(cfg.mask_pos)
    scale = float(D) ** -0.5
    HW = H * HD                      # width of all-heads V
    VCH = (HW + 511) // 512          # 512-wide chunks of it

    nc = bacc.Bacc("TRN2", target_bir_lowering=False, debug=False)
    dp = nc.declare_dram_parameter
    x_dec = dp("x_dec", [T, D], f32, isOutput=False)
    x_enc = dp("x_enc", [T, D], f32, isOutput=False)
    x_own = dp("x_own", [OWN, D], f32, isOutput=False)
    wq_sa = dp("wq_sa", [PAIRS, D, P], f32r, isOutput=False)
    wk_sa = dp("wk_sa", [PAIRS, D, P], f32r, isOutput=False)
    wv_sa = dp("wv_sa", [D, HW], f32r, isOutput=False)
    wo_sa = dp("wo_sa", [D, D], f32r, isOutput=False)
    bo_sa = dp("bo_sa", [D], f32, isOutput=False)
    wq_ca = dp("wq_ca", [PAIRS, D, P], f32r, isOutput=False)
    wk_ca = dp("wk_ca", [PAIRS, D, P], f32r, isOutput=False)
    wv_ca = dp("wv_ca", [D, HW], f32r, isOutput=False)
    wo_ca = dp("wo_ca", [D, D], f32r, isOutput=False)
    bo_ca = dp("bo_ca", [D], f32, isOutput=False)
    w1 = dp("w1", [D, FF], f32r, isOutput=False)
    b1 = dp("b1", [FF], f32, isOutput=False)
    w2 = dp("w2", [FF, D], f32r, isOutput=False)
    b2 = dp("b2", [D], f32, isOutput=False)
    masks = dp("masks", [P, NACT * P], bf16, isOutput=False)
    gbs = {}
    if with_gb:
        for n in ("g1", "be1", "g2", "be2", "g3", "be3"):
            gbs[n] = dp(n, [D], f32, isOutput=False)
    out = dp("out", [OWN, D], f32, isOutput=True)

    vsc_sa = nc.dram_tensor("vsc_sa", [T, H * 65], f32r)
    vsc_ca = nc.dram_tensor("vsc_ca", [T, H * 65], f32r)

    r = lambda ap: ap.bitcast(f32r)

    from contextlib import ExitStack
    with tile.TileContext(nc) as tc:
        with ExitStack() as _ctx:
            _ctx.enter_context(nc.allow_low_precision(
                reason="float32r matmul inputs (fp32r rounds ~fp32)"))
            _pool = lambda nm, bufs, **kw: _ctx.enter_context(
                tc.tile_pool(name=nm, bufs=bufs, **kw))
            constp = _pool("constp", 1)
            xTp = _pool("xTp", 1)
            lnqp = _pool("lnqp", 1)
            ktpp = _pool("ktpp", 2)
            qTp = _pool("qTp", 8)
            vstp = _pool("vstp", 2)
            avpp = _pool("avpp", 8)
            residp = _pool("residp", 4)
            pbp = _pool("pbp", 2)
            ldp = _pool("ldp", 2)
            evp = _pool("evp", 3)
            wpairp = _pool("wpairp", 3)
            wbigp = _pool("wbigp", 2)
            smallp = _pool("smallp", 8)
            bp = _pool("bp", 1)
            scps = _pool("scps", 2, space="PSUM")
            avps = _pool("avps", 2, space="PSUM")
            linps = _pool("linps", 2, space="PSUM")
            # ---------------- constants ----------------
            ident = constp.tile([P, P], f32, tag="ident")
            make_identity(nc, ident[:])
            identb = constp.tile([P, P], bf16, tag="identb")
            make_identity(nc, identb[:])
            ones65f = constp.tile([65, HD], f32, tag="ones65f")
            nc.any.memset(ones65f[:], 1.0)
            ones65 = constp.tile([65, HD], f32r, tag="ones65")
            nc.vector.tensor_copy(ones65[:], ones65f[:])
            ones8 = constp.tile([P, 8], f32, tag="ones8")
            nc.any.memset(ones8[:], 1.0)
            epst = constp.tile([P, 1], f32, tag="epst")
            nc.any.memset(epst[:], EPS)
            maskt = constp.tile([P, NACT * P], bf16, tag="maskt")
            nc.sync.dma_start(maskt[:], masks[:])

            def load_bias_T(dram, n):
                """[n*P] DRAM vector -> [P, n] tile (chunk c in column c)."""
                t = bp.tile([P, n], f32, tag=dram.tensor.name + "_t")
                nc.sync.dma_start(t[:], dram[:].rearrange("(d p) -> p d", p=P))
                return t

            gb_tiles = {}
            if with_gb:
                for gk, bk, key in (("g1", "be1", 1), ("g2", "be2", 2),
                                    ("g3", "be3", 3)):
                    gb_tiles[key] = (load_bias_T(gbs[gk][:], DC),
                                    load_bias_T(gbs[bk][:], DC))
            bo_sa_t = load_bias_T(bo_sa[:], DC)
            bo_ca_t = load_bias_T(bo_ca[:], DC)
            b1_t = load_bias_T(b1[:], FC)
            b2_t = load_bias_T(b2[:], DC)

            # ---------------- helpers ----------------
            def emit_ln(xt, gbkey, inplace=True):
                """LayerNorm over D (gamma/beta folded at transpose-evict)."""
                nch = (D + 511) // 512
                st6 = smallp.tile([P, nch * 6], f32, tag="st6")
                for i in range(nch):
                    c0, c1 = i * 512, min(D, (i + 1) * 512)
                    nc.vector.bn_stats(st6[:, i * 6:(i + 1) * 6], xt[:, c0:c1])
                mv = smallp.tile([P, 2], f32, tag="mv")
                nc.vector.bn_aggr(mv[:], st6[:].rearrange("p (a b) -> p a b",
                                                          b=6))
                std = smallp.tile([P, 1], f32, tag="std")
                nc.scalar.activation(std[:], mv[:, 1:2], AF.Sqrt, bias=epst[:])
                rstd = smallp.tile([P, 1], f32, tag="rstd")
                nc.vector.reciprocal(rstd[:], std[:])
                mrs = smallp.tile([P, 1], f32, tag="mrs")
                nc.vector.tensor_mul(mrs[:], mv[:, 0:1], rstd[:])
                nmrs = smallp.tile([P, 1], f32, tag="nmrs")
                nc.vector.tensor_scalar_mul(nmrs[:], mrs[:], -1.0)
                if inplace:
                    lnt = xt
                else:
                    lnt = ldp.tile([P, D], f32, tag="ld")
                nc.scalar.activation(lnt[:], xt[:], AF.Identity,
                                     bias=nmrs[:], scale=rstd[:])
                return lnt

            def emit_transposes(src, dst_view, gbkey):
                """Transpose [P, D] src into dst_view [P, DC, P] (d-major),
                packing 4 transposes per PSUM tile."""
                for g0 in range(0, DC, 4):
                    gn = min(4, DC - g0)
                    ps = linps.tile([P, 512], f32, tag="lin")
                    for i in range(gn):
                        d = g0 + i
                        nc.tensor.transpose(ps[:, i * P:(i + 1) * P],
                                            src[:, d * P:(d + 1) * P],
                                            ident[:])
                    if with_gb and gbkey is not None:
                        gt, bt = gb_tiles[gbkey]
                        for i in range(gn):
                            d = g0 + i
                            nc.scalar.activation(
                                dst_view[:, d, :], ps[:, i * P:(i + 1) * P],
                                AF.Identity, bias=bt[:, d:d + 1],
                                scale=gt[:, d:d + 1])
                    else:
                        nc.vector.tensor_copy(dst_view[:, g0:g0 + gn, :],
                                              ps[:, :gn * P])

            def ln_transpose_stream(src_dram, row0, nrows, xT, gbkey):
                """Stream [nrows, D] from DRAM (rows row0..), LN if gbkey,
                transpose into mega-tile xT ([P, DC*TH], token col = local)."""
                xTv = xT[:].rearrange("p (d t) -> p d t", t=TH)
                for tb in range(nrows // P):
                    xt = ldp.tile([P, D], f32, tag="ld")
                    nc.sync.dma_start(
                        xt[:], src_dram[row0 + tb * P:row0 + (tb + 1) * P, :])
                    lnt = emit_ln(xt, gbkey) if gbkey is not None else xt
                    emit_transposes(lnt, xTv[:, :, tb * P:(tb + 1) * P], gbkey)

            def emit_pair_proj(w_dram, pair, rhs_slices, rhs_w, dst, dst_c0):
                """dst[:, c] = w_pair.T @ rhs ([128=2 heads] rows), contracting
                D in 128-chunks. rhs_slices(d, c0, cw) -> AP."""
                wt = wpairp.tile([P, DC * P], f32r, tag="wpair")
                nc.sync.dma_start(
                    wt[:].rearrange("p (d c) -> p d c", c=P),
                    w_dram[pair].rearrange("(d p) c -> p d c", p=P))
                for c0 in range(0, rhs_w, 512):
                    cw = min(512, rhs_w - c0)
                    ps = linps.tile([P, 512], f32, tag="lin")
                    for d in range(DC):
                        nc.tensor.matmul(ps[:, :cw], r(wt[:, d * P:(d + 1) * P]),
                                         r(rhs_slices(d, c0, cw)),
                                         start=(d == 0), stop=(d == DC - 1))
                    nc.vector.tensor_copy(dst[:, dst_c0 + c0:dst_c0 + c0 + cw],
                                          ps[:, :cw])

            def emit_v_to_scratch(wv_dram, xT, half, vsc):
                """V in natural layout for all heads -> DRAM scratch; each
                head gets 65 columns with col 64 = 1.0 (softmax denominator
                rides the AV matmul for free)."""
                for nch in range(VCH):
                    c0, cw = nch * 512, min(512, HW - nch * 512)
                    nh = cw // HD
                    wvh = wbigp.tile([P, DC * 512], f32r, tag="wbig",
                                     name=f"wvh{half}{nch}")
                    nc.sync.dma_start(
                        wvh[:, :DC * cw].rearrange("p (d c) -> p d c", c=cw),
                        wv_dram[:, c0:c0 + cw].rearrange("(d p) c -> p d c",
                                                         p=P))
                    for tb in range(TH // P):
                        ps = linps.tile([P, 512], f32, tag="lin")
                        for d in range(DC):
                            nc.tensor.matmul(
                                ps[:, :cw],
                                r(xT[:, d * TH + tb * P:d * TH + (tb + 1) * P]),
                                r(wvh[:, d * cw:(d + 1) * cw]),
                                start=(d == 0), stop=(d == DC - 1))
                        ev = evp.tile([P, 8 * 65], f32r, tag="ev")
                        evv = ev[:, :nh * 65].rearrange("p (h c) -> p h c",
                                                        c=65)
                        nc.vector.tensor_copy(
                            evv[:, :, 64:65],
                            ones8[:, :nh].unsqueeze(2))
                        nc.scalar.activation(
                            evv[:, :, 0:HD],
                            ps[:, :cw].rearrange("p (h c) -> p h c", c=HD),
                            AF.Copy)
                        row0 = half * TH + tb * P
                        nc.sync.dma_start(
                            vsc[row0:row0 + P,
                                nch * 8 * 65:nch * 8 * 65 + nh * 65],
                            ev[:, :nh * 65])

            def emit_kt_pair(w_dram, pair, xT):
                """K^T for one head pair from transposed activations."""
                kt = ktpp.tile([P, TH], f32r, tag="ktp")
                emit_pair_proj(w_dram, pair,
                               lambda d, c0, cw: xT[:, d * TH + c0:
                                                    d * TH + c0 + cw],
                               TH, kt, 0)
                return kt

            def emit_attention(qT, kt, vsc, half, pair, avst, use_masks):
                """One half-T of attention, both heads of a pair.
                qT: [P, OWN] (rows 0:64 head A, 64:128 head B).
                kt: [P, TH]. avst: dict with rolling 'psum' + 'sbuf' [P,2*OWN]
                partial accumulator ([O^T;denom] per head in column halves)."""
                vtt = vstp.tile([P, KBH * 130], f32r, tag="vst")
                nc.sync.dma_start(
                    vtt[:].rearrange("p (kl c) -> p kl c", c=130),
                    vsc[half * TH:(half + 1) * TH,
                        pair * 130:(pair + 1) * 130]
                    .rearrange("(kl p) c -> p kl c", p=P))
                NG = (KBH + 1) // 2
                for hh in range(2):
                    hb = hh * HD
                    for g in range(NG):
                        kls = [kl for kl in (2 * g, 2 * g + 1) if kl < KBH]
                        sc = scps.tile([P, 2 * OWN], f32, tag="sc")
                        for i, kl in enumerate(kls):
                            kg = half * KBH + kl
                            mss = [s for s in range(NQB)
                                   if use_masks and (s, kg) in cfg.mask_idx]
                            nc.tensor.matmul(
                                sc[:, i * OWN:(i + 1) * OWN],
                                r(kt[hb:hb + HD, kl * P:(kl + 1) * P]),
                                r(qT[hb:hb + HD, :]),
                                start=True, stop=(not mss))
                            for n, s in enumerate(mss):
                                idx = cfg.mask_idx[(s, kg)]
                                nc.tensor.matmul(
                                    sc[:, i * OWN + s * P:
                                       i * OWN + (s + 1) * P],
                                    identb[:],
                                    maskt[:, idx * P:(idx + 1) * P],
                                    start=False, stop=(n == len(mss) - 1))
                        pb = pbp.tile([P, 2 * OWN], f32r, tag="pb")
                        nc.scalar.activation(pb[:, :len(kls) * OWN],
                                             sc[:, :len(kls) * OWN],
                                             AF.Exp, scale=scale)
                        for i, kl in enumerate(kls):
                            nc.tensor.matmul(
                                avst["psum"][:],
                                r(vtt[:, kl * 130 + hh * 65:
                                      kl * 130 + (hh + 1) * 65]),
                                r(pb[:, i * OWN:(i + 1) * OWN]),
                                start=(g == 0 and i == 0),
                                stop=(g == NG - 1 and i == len(kls) - 1))
                    dst = avst["sbuf"][0:65, hh * OWN:(hh + 1) * OWN]
                    if half == 0:
                        nc.vector.tensor_copy(dst, avst["psum"][:])
                    else:
                        nc.vector.tensor_add(dst, avst["psum"][:], dst)
                    if not (half == 1 and hh == 1):
                        avst["psum"] = avps.tile([65, OWN], f32, tag="av",
                                                 name=f"avps_{pair}_{half}_{hh}")

            def emit_normalize(avp_sb, hh):
                """O^T /= denominator row, in place in the sbuf partial."""
                cs = hh * OWN
                rec = evp.tile([P, 512], f32r, tag="ev")
                nc.vector.reciprocal(rec[64:65, :OWN],
                                     avp_sb[64:65, cs:cs + OWN])
                bc = avps.tile([64, OWN], f32, tag="av")
                nc.tensor.matmul(bc[:], r(ones65[64:65, :]),
                                 r(rec[64:65, :OWN]), start=True, stop=True)
                bcs = evp.tile([P, 512], f32, tag="ev")
                nc.scalar.activation(bcs[0:64, :OWN], bc[:], AF.Copy)
                nc.vector.tensor_mul(avp_sb[0:64, cs:cs + OWN],
                                     avp_sb[0:64, cs:cs + OWN],
                                     bcs[0:64, :OWN])

            def emit_oproj_residual(wo_dram, bo_t, avp_list, res_tiles):
                """res += transpose(Wo^T @ O^T + bo)   (residual in place)."""
                for m in range(DC):
                    wot = wbigp.tile([64, 2 * DC * P], f32r, tag="wbig",
                                     name=f"wot{m}")
                    nc.sync.dma_start(
                        wot[:].rearrange("p (a c) -> p a c", c=P),
                        wo_dram[:, m * P:(m + 1) * P]
                        .rearrange("(a p) c -> p a c", p=64))
                    ps = linps.tile([P, 512], f32, tag="lin")
                    for n in range(2 * DC):
                        pair, hh = n // 2, n % 2
                        nc.tensor.matmul(
                            ps[:, :OWN], r(wot[:, n * P:(n + 1) * P]),
                            r(avp_list[pair][0:64, hh * OWN:(hh + 1) * OWN]),
                            start=(n == 0), stop=(n == 2 * DC - 1))
                    ev = evp.tile([P, 512], f32, tag="ev")
                    nc.scalar.activation(ev[:, :OWN], ps[:, :OWN], AF.Identity,
                                         bias=bo_t[:, m:m + 1])
                    ps2 = linps.tile([P, 512], f32, tag="lin")
                    for s in range(NQB):
                        nc.tensor.transpose(ps2[:, s * P:(s + 1) * P],
                                            ev[:, s * P:(s + 1) * P], ident[:])
                    for s in range(NQB):
                        nc.vector.tensor_add(
                            res_tiles[s][:, m * P:(m + 1) * P],
                            ps2[:, s * P:(s + 1) * P],
                            res_tiles[s][:, m * P:(m + 1) * P])

            def emit_lnq(res_tiles_or_dram, gbkey, from_dram):
                """LN own tokens + transpose -> [P, DC*OWN] mega-tile."""
                lnq = lnqp.tile([P, DC * OWN], f32r, tag="lnq")
                lnqv = lnq[:].rearrange("p (d t) -> p d t", t=OWN)
                for s in range(NQB):
                    if from_dram:
                        xt = ldp.tile([P, D], f32, tag="ld")
                        nc.sync.dma_start(
                            xt[:], res_tiles_or_dram[s * P:(s + 1) * P, :])
                    else:
                        xt = res_tiles_or_dram[s]
                    lnt = emit_ln(xt[:], gbkey, inplace=from_dram)
                    emit_transposes(lnt, lnqv[:, :, s * P:(s + 1) * P], gbkey)
                return lnq

            # ================= pipeline =================
            res = []
            for s in range(NQB):
                t = residp.tile([P, D], f32, tag="resid")
                nc.sync.dma_start(t[:], x_own[s * P:(s + 1) * P, :])
                res.append(t)

            # own-token LN1 -> Q_sa^T
            lnq1 = emit_lnq(x_own, 1, True)
            qsaT = []
            for pair in range(PAIRS):
                qt = qTp.tile([P, OWN], f32r, tag="qT")
                emit_pair_proj(
                    wq_sa, pair,
                    lambda d, c0, cw: lnq1[:, d * OWN + c0:d * OWN + c0 + cw],
                    OWN, qt, 0)
                qsaT.append(qt)

            # SA attention in two half-T passes
            av_sa = [avpp.tile([P, 2 * OWN], f32r, tag="avp", name=f"av_sa{_pp}")
                     for _pp in range(PAIRS)]
            avst_sa = {}
            for half in range(2):
                xT = xTp.tile([P, DC * TH], f32r, tag="xT")
                ln_transpose_stream(x_dec, half * TH, TH, xT, 1)
                emit_v_to_scratch(wv_sa, xT, half, vsc_sa)
                for pair in range(PAIRS):
                    kt = emit_kt_pair(wk_sa, pair, xT)
                    if half == 0:
                        avst_sa[pair] = {
                            "psum": avps.tile([65, OWN], f32, tag="av",
                                              name=f"avps_sa{pair}"),
                            "sbuf": av_sa[pair]}
                    emit_attention(qsaT[pair], kt, vsc_sa, half, pair,
                                   avst_sa[pair], True)
                    if half == 1:
                        emit_normalize(av_sa[pair], 0)
                        emit_normalize(av_sa[pair], 1)
            emit_oproj_residual(wo_sa, bo_sa_t, av_sa, res)     # res -> x2

            # LN2 -> Q_ca^T
            lnq2 = emit_lnq(res, 2, False)
            qcaT = []
            for pair in range(PAIRS):
                qt = qTp.tile([P, OWN], f32r, tag="qT")
                emit_pair_proj(
                    wq_ca, pair,
                    lambda d, c0, cw: lnq2[:, d * OWN + c0:d * OWN + c0 + cw],
                    OWN, qt, 0)
                qcaT.append(qt)

            # CA attention (raw encoder K/V, no masks)
            av_ca = [avpp.tile([P, 2 * OWN], f32r, tag="avp", name=f"av_ca{_pp}")
                     for _pp in range(PAIRS)]
            avst_ca = {}
            for half in range(2):
                xT = xTp.tile([P, DC * TH], f32r, tag="xT")
                ln_transpose_stream(x_enc, half * TH, TH, xT, None)
                emit_v_to_scratch(wv_ca, xT, half, vsc_ca)
                for pair in range(PAIRS):
                    kt = emit_kt_pair(wk_ca, pair, xT)
                    if half == 0:
                        avst_ca[pair] = {
                            "psum": avps.tile([65, OWN], f32, tag="av",
                                              name=f"avps_ca{pair}"),
                            "sbuf": av_ca[pair]}
                    emit_attention(qcaT[pair], kt, vsc_ca, half, pair,
                                   avst_ca[pair], False)
                    if half == 1:
                        emit_normalize(av_ca[pair], 0)
                        emit_normalize(av_ca[pair], 1)
            emit_oproj_residual(wo_ca, bo_ca_t, av_ca, res)     # res -> x3

            # LN3 -> FFN
            lnq3 = emit_lnq(res, 3, False)
            y2T = [qTp.tile([P, OWN], f32, tag="qT", name=f"y2T{_m}") for _m in range(DC)]
            FG = FC // 2
            for fg in range(2):
                rT = xTp.tile([P, DC * TH], f32r, tag="xT")
                for fi in range(FG):
                    f = fg * FG + fi
                    w1t = wpairp.tile([P, DC * P], f32r, tag="wpair",
                                      name=f"w1t{f}")
                    nc.sync.dma_start(
                        w1t[:].rearrange("p (d c) -> p d c", c=P),
                        w1[:, f * P:(f + 1) * P]
                        .rearrange("(d p) c -> p d c", p=P))
                    ps = linps.tile([P, 512], f32, tag="lin")
                    for d in range(DC):
                        nc.tensor.matmul(
                            ps[:, :OWN], r(w1t[:, d * P:(d + 1) * P]),
                            r(lnq3[:, d * OWN:(d + 1) * OWN]),
                            start=(d == 0), stop=(d == DC - 1))
                    nc.scalar.activation(rT[:, fi * OWN:(fi + 1) * OWN],
                                         ps[:, :OWN], AF.Relu,
                                         bias=b1_t[:, f:f + 1])
                for m in range(DC):
                    w2t = wbigp.tile([P, FG * P], f32r, tag="wbig",
                                     name=f"w2t{fg}{m}")
                    nc.sync.dma_start(
                        w2t[:].rearrange("p (a c) -> p a c", c=P),
                        w2[fg * FG * P:(fg + 1) * FG * P, m * P:(m + 1) * P]
                        .rearrange("(a p) c -> p a c", p=P))
                    ps = linps.tile([P, 512], f32, tag="lin")
                    for fi in range(FG):
                        nc.tensor.matmul(
                            ps[:, :OWN], r(w2t[:, fi * P:(fi + 1) * P]),
                            r(rT[:, fi * OWN:(fi + 1) * OWN]),
                            start=(fi == 0), stop=(fi == FG - 1))
                    if fg == 0:
                        nc.scalar.activation(y2T[m][:], ps[:, :OWN],
                                             AF.Identity,
                                             bias=b2_t[:, m:m + 1])
                    else:
                        nc.vector.tensor_add(y2T[m][:], ps[:, :OWN],
                                             y2T[m][:])

            for m in range(DC):
                ps2 = linps.tile([P, 512], f32, tag="lin")
                for s in range(NQB):
                    nc.tensor.transpose(ps2[:, s * P:(s + 1) * P],
                                        y2T[m][:, s * P:(s + 1) * P], ident[:])
                for s in range(NQB):
                    nc.vector.tensor_add(res[s][:, m * P:(m + 1) * P],
                                         ps2[:, s * P:(s + 1) * P],
                                         res[s][:, m * P:(m + 1) * P])
            for s in range(NQB):
                nc.sync.dma_start(out[s * P:(s + 1) * P, :], res[s][:])

    nc.compile()
    return nc


def own_token_rows(cfg, j):
    return np.concatenate(
        [np.arange(P * (cfg.NQB * s + j), P * (cfg.NQB * s + j) + P)
         for s in range(cfg.NQB)])


def prep_core_inputs(cfg, inputs, core):
    """Host-side slicing/packing for one core."""
    D, H = cfg.D, cfg.H
    b, j = core // 4, core % 4
    a = lambda x: np.asarray(x)
    f32c = lambda x: np.ascontiguousarray(a(x), dtype=np.float32)
    pack_pairs = lambda w: np.ascontiguousarray(np.stack(
        [np.concatenate([a(w)[2 * p], a(w)[2 * p + 1]], axis=1)
         for p in range(cfg.PAIRS)]), dtype=np.float32)
    vall = lambda w: np.ascontiguousarray(
        a(w).transpose(1, 0, 2).reshape(D, H * HD), dtype=np.float32)

    rows = own_token_rows(cfg, j)
    return {
        "x_dec": f32c(a(inputs["decoder_x"])[b]),
        "x_enc": f32c(a(inputs["encoder_x"])[b]),
        "x_own": f32c(a(inputs["decoder_x"])[b][rows]),
        "wq_sa": pack_pairs(inputs["Wq_sa"]),
        "wk_sa": pack_pairs(inputs["Wk_sa"]),
        "wv_sa": vall(inputs["Wv_sa"]),
        "wo_sa": f32c(inputs["Wo_sa"]),
        "bo_sa": f32c(inputs["bo_sa"]),
        "wq_ca": pack_pairs(inputs["Wq_ca"]),
        "wk_ca": pack_pairs(inputs["Wk_ca"]),
        "wv_ca": vall(inputs["Wv_ca"]),
        "wo_ca": f32c(inputs["Wo_ca"]),
        "bo_ca": f32c(inputs["bo_ca"]),
        "w1": f32c(inputs["W1"]),
        "b1": f32c(inputs["b1"]),
        "w2": f32c(inputs["W2"]),
        "b2": f32c(inputs["b2"]),
        "masks": build_masks(cfg, j).astype(ml_dtypes.bfloat16),
    }, rows


def gb_trivial(inputs):
    return all(np.allclose(np.asarray(inputs[g]), 1.0)
               for g in ("g1", "g2", "g3")) and \
           all(np.allclose(np.asarray(inputs[b]), 0.0)
               for b in ("be1", "be2", "be3"))


def run(inputs, trace=False, **rk):
    """Build + run on 8 cores; returns (full_output, BassKernelResults)."""
    from concourse.bass_utils import run_bass_kernel_spmd

    cfg = Cfg()
    with_gb = not gb_trivial(inputs)
    nc = build_nc(cfg, with_gb)

    in_maps, rows_all = [], []
    for core in range(8):
        im, rows = prep_core_inputs(cfg, inputs, core)
        if with_gb:
            for n in ("g1", "be1", "g2", "be2", "g3", "be3"):
                im[n] = np.ascontiguousarray(np.asarray(inputs[n]),
                                             dtype=np.float32)
        in_maps.append(im)
        rows_all.append(rows)

    res = run_bass_kernel_spmd(nc, in_maps, list(range(8)), trace=trace, **rk)
    full = np.zeros((2, cfg.T, cfg.D), np.float32)
    for core in range(8):
        full[core // 4][rows_all[core]] = res.results[core]["out"]
    return full, res


def kernel(**inputs) -> np.ndarray:
    return run(inputs)[0]



# revision 18
# speedup vs baseline: 1.0893x; 1.0146x over previous
"""Trainium2 Bass kernel for a transformer decoder block (self-attn + cross-attn + FFN).

Sharding: zero-collective data parallelism over tokens. 8 cores; core c handles
batch b = c//4 and the 4 query blocks {4s + (c%4) : s in 0..3} of 128 tokens
each. Each core redundantly computes full-sequence K/V projections (cheaper
than on-chip collectives at this size) and everything else only for its own
tokens. The causal-attention structure is uniform across cores (one SPMD
program); per-core causal masks arrive as input data and are added to scores
with identity matmuls.

Layouts: activations are kept "transposed" ([feature, token]) for matmuls so
weights are always the stationary operand; LayerNorm/softmax-denominator/
residual work happens in [token, feature] layout; PE transposes convert.
Scores are computed as S^T = K^T.T @ Q^T ([key, query]), so the AV matmul
lhsT=[V|ones] both contracts keys and produces the softmax denominator for
free in PSUM row 64.
"""
import sys
import numpy as np
import ml_dtypes

for _p in ('/opt/trn_rl_repo',):
    if _p not in sys.path:
        sys.path.append(_p)

import concourse.bass as bass
import concourse.tile as tile
from concourse import bacc, mybir
from concourse.masks import make_identity

P = 128
HD = 64
EPS = 1e-5
NEG = -1e9

f32 = mybir.dt.float32
f32r = mybir.dt.float32r
bf16 = mybir.dt.bfloat16
AF = mybir.ActivationFunctionType


class Cfg:
    def __init__(self, T=2048, D=1024, H=16, FF=4096):
        self.T, self.D, self.H, self.FF = T, D, H, FF
        self.OWN = T // 4          # tokens per core
        self.NQB = self.OWN // P   # own q-blocks (128 each)
        self.DC = D // P           # D chunks
        self.FC = FF // P          # FFN chunks
        self.PAIRS = H // 2
        self.KB = T // P           # key blocks (global)
        self.TH = T // 2           # tokens per half
        self.KBH = self.KB // 2    # key blocks per half
        assert self.OWN <= 512 and self.T % 256 == 0 and D % P == 0
        assert H % 2 == 0 and self.FC % 2 == 0
        # query-column offset per key block (clamped to 256 so f32r
        # matmuls keep >=256-wide moving dim); queries below off are never
        # visible for block k on any core, so score/exp/AV skip them.
        self.off = {k: min(k // self.NQB, 2) * P for k in range(self.KB)}
        # active mask positions among computed (s, k) regions
        self.mask_pos = [(s, k) for s in range(self.NQB)
                         for k in range(self.NQB * s, self.KB)
                         if s * P >= self.off[k]]
        self.mask_idx = {sk: i for i, sk in enumerate(self.mask_pos)}


def build_masks(cfg, j):
    """Additive causal mask tiles for core j: [P, n_active*P] f32."""
    m = np.zeros((len(cfg.mask_pos), P, P), np.float32)
    for i, (s, k) in enumerate(cfg.mask_pos):
        g = cfg.NQB * s + j
        if k < g:
            continue                      # fully visible: additive zero
        elif k == g:
            pidx = np.arange(P)[:, None]  # key within block
            fidx = np.arange(P)[None, :]  # query within block
            m[i] = np.where(pidx <= fidx, 0.0, NEG)
        else:
            m[i] = NEG
    return np.ascontiguousarray(m.transpose(1, 0, 2).reshape(P, -1))


def build_nc(cfg, with_gb):
    T, D, H, FF = cfg.T, cfg.D, cfg.H, cfg.FF
    OWN, NQB, DC, FC = cfg.OWN, cfg.NQB, cfg.DC, cfg.FC
    PAIRS, KB, TH, KBH = cfg.PAIRS, cfg.KB, cfg.TH, cfg.KBH
    NACT = len(cfg.mask_pos)
    scale = float(D) ** -0.5
    HW = H * HD                      # width of all-heads V
    VCH = (HW + 511) // 512          # 512-wide chunks of it

    nc = bacc.Bacc("TRN2", target_bir_lowering=False, debug=False)
    dp = nc.declare_dram_parameter
    x_dec = dp("x_dec", [T, D], f32, isOutput=False)
    x_enc = dp("x_enc", [T, D], f32, isOutput=False)
    x_own = dp("x_own", [OWN, D], f32, isOutput=False)
    wq_sa = dp("wq_sa", [PAIRS, D, P], f32r, isOutput=False)
    wk_sa = dp("wk_sa", [PAIRS, D, P], f32r, isOutput=False)
    wv_sa = dp("wv_sa", [D, HW], f32r, isOutput=False)
    wo_sa = dp("wo_sa", [D, D], f32r, isOutput=False)
    bo_sa = dp("bo_sa", [D], f32, isOutput=False)
    wq_ca = dp("wq_ca", [PAIRS, D, P], f32r, isOutput=False)
    wk_ca = dp("wk_ca", [PAIRS, D, P], f32r, isOutput=False)
    wv_ca = dp("wv_ca", [D, HW], f32r, isOutput=False)
    wo_ca = dp("wo_ca", [D, D], f32r, isOutput=False)
    bo_ca = dp("bo_ca", [D], f32, isOutput=False)
    w1 = dp("w1", [D, FF], f32r, isOutput=False)
    b1 = dp("b1", [FF], f32, isOutput=False)
    w2 = dp("w2", [FF, D], f32r, isOutput=False)
    b2 = dp("b2", [D], f32, isOutput=False)
    masks = dp("masks", [P, NACT * P], bf16, isOutput=False)
    gbs = {}
    if with_gb:
        for n in ("g1", "be1", "g2", "be2", "g3", "be3"):
            gbs[n] = dp(n, [D], f32, isOutput=False)
    out = dp("out", [OWN, D], f32, isOutput=True)

    vsc_sa = nc.dram_tensor("vsc_sa", [T, H * 65], f32r)
    vsc_ca = nc.dram_tensor("vsc_ca", [T, H * 65], f32r)

    r = lambda ap: ap.bitcast(f32r)

    from contextlib import ExitStack
    with tile.TileContext(nc) as tc:
        with ExitStack() as _ctx:
            _ctx.enter_context(nc.allow_low_precision(
                reason="float32r matmul inputs (fp32r rounds ~fp32)"))
            _pool = lambda nm, bufs, **kw: _ctx.enter_context(
                tc.tile_pool(name=nm, bufs=bufs, **kw))
            constp = _pool("constp", 1)
            xTp = _pool("xTp", 1)
            lnqp = _pool("lnqp", 1)
            ktpp = _pool("ktpp", 2)
            qTp = _pool("qTp", 8)
            vstp = _pool("vstp", 2)
            avpp = _pool("avpp", 8)
            residp = _pool("residp", 4)
            pbp = _pool("pbp", 2)
            ldp = _pool("ldp", 2)
            evp = _pool("evp", 3)
            wpairp = _pool("wpairp", 3)
            wbigp = _pool("wbigp", 2)
            smallp = _pool("smallp", 8)
            bp = _pool("bp", 1)
            scps = _pool("scps", 2, space="PSUM")
            avps = _pool("avps", 2, space="PSUM")
            linps = _pool("linps", 2, space="PSUM")
            # ---------------- constants ----------------
            ident = constp.tile([P, P], f32, tag="ident")
            make_identity(nc, ident[:])
            identb = constp.tile([P, P], bf16, tag="identb")
            make_identity(nc, identb[:])
            ones65f = constp.tile([65, HD], f32, tag="ones65f")
            nc.any.memset(ones65f[:], 1.0)
            ones65 = constp.tile([65, HD], f32r, tag="ones65")
            nc.vector.tensor_copy(ones65[:], ones65f[:])
            ones8 = constp.tile([P, 8], f32, tag="ones8")
            nc.any.memset(ones8[:], 1.0)
            epst = constp.tile([P, 1], f32, tag="epst")
            nc.any.memset(epst[:], EPS)
            maskt = constp.tile([P, NACT * P], bf16, tag="maskt")
            nc.sync.dma_start(maskt[:], masks[:])

            def load_bias_T(dram, n):
                """[n*P] DRAM vector -> [P, n] tile (chunk c in column c)."""
                t = bp.tile([P, n], f32, tag=dram.tensor.name + "_t")
                nc.sync.dma_start(t[:], dram[:].rearrange("(d p) -> p d", p=P))
                return t

            gb_tiles = {}
            if with_gb:
                for gk, bk, key in (("g1", "be1", 1), ("g2", "be2", 2),
                                    ("g3", "be3", 3)):
                    gb_tiles[key] = (load_bias_T(gbs[gk][:], DC),
                                    load_bias_T(gbs[bk][:], DC))
            bo_sa_t = load_bias_T(bo_sa[:], DC)
            bo_ca_t = load_bias_T(bo_ca[:], DC)
            b1_t = load_bias_T(b1[:], FC)
            b2_t = load_bias_T(b2[:], DC)

            # ---------------- helpers ----------------
            def emit_ln(xt, gbkey, inplace=True):
                """LayerNorm over D (gamma/beta folded at transpose-evict)."""
                nch = (D + 511) // 512
                st6 = smallp.tile([P, nch * 6], f32, tag="st6")
                for i in range(nch):
                    c0, c1 = i * 512, min(D, (i + 1) * 512)
                    nc.vector.bn_stats(st6[:, i * 6:(i + 1) * 6], xt[:, c0:c1])
                mv = smallp.tile([P, 2], f32, tag="mv")
                nc.vector.bn_aggr(mv[:], st6[:].rearrange("p (a b) -> p a b",
                                                          b=6))
                std = smallp.tile([P, 1], f32, tag="std")
                nc.scalar.activation(std[:], mv[:, 1:2], AF.Sqrt, bias=epst[:])
                rstd = smallp.tile([P, 1], f32, tag="rstd")
                nc.vector.reciprocal(rstd[:], std[:])
                mrs = smallp.tile([P, 1], f32, tag="mrs")
                nc.vector.tensor_mul(mrs[:], mv[:, 0:1], rstd[:])
                nmrs = smallp.tile([P, 1], f32, tag="nmrs")
                nc.vector.tensor_scalar_mul(nmrs[:], mrs[:], -1.0)
                if inplace:
                    lnt = xt
                else:
                    lnt = ldp.tile([P, D], f32, tag="ld")
                nc.scalar.activation(lnt[:], xt[:], AF.Identity,
                                     bias=nmrs[:], scale=rstd[:])
                return lnt

            def emit_transposes(src, dst_view, gbkey):
                """Transpose [P, D] src into dst_view [P, DC, P] (d-major),
                packing 4 transposes per PSUM tile."""
                for g0 in range(0, DC, 4):
                    gn = min(4, DC - g0)
                    ps = linps.tile([P, 512], f32, tag="lin")
                    for i in range(gn):
                        d = g0 + i
                        nc.tensor.transpose(ps[:, i * P:(i + 1) * P],
                                            src[:, d * P:(d + 1) * P],
                                            ident[:])
                    if with_gb and gbkey is not None:
                        gt, bt = gb_tiles[gbkey]
                        for i in range(gn):
                            d = g0 + i
                            nc.scalar.activation(
                                dst_view[:, d, :], ps[:, i * P:(i + 1) * P],
                                AF.Identity, bias=bt[:, d:d + 1],
                                scale=gt[:, d:d + 1])
                    else:
                        nc.vector.tensor_copy(dst_view[:, g0:g0 + gn, :],
                                              ps[:, :gn * P])

            def ln_transpose_stream(src_dram, row0, nrows, xT, gbkey):
                """Stream [nrows, D] from DRAM (rows row0..), LN if gbkey,
                transpose into mega-tile xT ([P, DC*TH], token col = local)."""
                xTv = xT[:].rearrange("p (d t) -> p d t", t=TH)
                for tb in range(nrows // P):
                    xt = ldp.tile([P, D], f32, tag="ld")
                    nc.sync.dma_start(
                        xt[:], src_dram[row0 + tb * P:row0 + (tb + 1) * P, :])
                    lnt = emit_ln(xt, gbkey) if gbkey is not None else xt
                    emit_transposes(lnt, xTv[:, :, tb * P:(tb + 1) * P], gbkey)

            def emit_pair_proj(w_dram, pair, rhs_slices, rhs_w, dst, dst_c0):
                """dst[:, c] = w_pair.T @ rhs ([128=2 heads] rows), contracting
                D in 128-chunks. rhs_slices(d, c0, cw) -> AP."""
                wt = wpairp.tile([P, DC * P], f32r, tag="wpair")
                nc.sync.dma_start(
                    wt[:].rearrange("p (d c) -> p d c", c=P),
                    w_dram[pair].rearrange("(d p) c -> p d c", p=P))
                for c0 in range(0, rhs_w, 512):
                    cw = min(512, rhs_w - c0)
                    ps = linps.tile([P, 512], f32, tag="lin")
                    for d in range(DC):
                        nc.tensor.matmul(ps[:, :cw], r(wt[:, d * P:(d + 1) * P]),
                                         r(rhs_slices(d, c0, cw)),
                                         start=(d == 0), stop=(d == DC - 1))
                    nc.vector.tensor_copy(dst[:, dst_c0 + c0:dst_c0 + c0 + cw],
                                          ps[:, :cw])

            def emit_v_to_scratch(wv_dram, xT, half, vsc):
                """V in natural layout for all heads -> DRAM scratch; each
                head gets 65 columns with col 64 = 1.0 (softmax denominator
                rides the AV matmul for free)."""
                for nch in range(VCH):
                    c0, cw = nch * 512, min(512, HW - nch * 512)
                    nh = cw // HD
                    wvh = wbigp.tile([P, DC * 512], f32r, tag="wbig",
                                     name=f"wvh{half}{nch}")
                    nc.sync.dma_start(
                        wvh[:, :DC * cw].rearrange("p (d c) -> p d c", c=cw),
                        wv_dram[:, c0:c0 + cw].rearrange("(d p) c -> p d c",
                                                         p=P))
                    for tb in range(TH // P):
                        ps = linps.tile([P, 512], f32, tag="lin")
                        for d in range(DC):
                            nc.tensor.matmul(
                                ps[:, :cw],
                                r(xT[:, d * TH + tb * P:d * TH + (tb + 1) * P]),
                                r(wvh[:, d * cw:(d + 1) * cw]),
                                start=(d == 0), stop=(d == DC - 1))
                        ev = evp.tile([P, 8 * 65], f32r, tag="ev")
                        evv = ev[:, :nh * 65].rearrange("p (h c) -> p h c",
                                                        c=65)
                        nc.vector.tensor_copy(
                            evv[:, :, 64:65],
                            ones8[:, :nh].unsqueeze(2))
                        nc.scalar.activation(
                            evv[:, :, 0:HD],
                            ps[:, :cw].rearrange("p (h c) -> p h c", c=HD),
                            AF.Copy)
                        row0 = half * TH + tb * P
                        nc.sync.dma_start(
                            vsc[row0:row0 + P,
                                nch * 8 * 65:nch * 8 * 65 + nh * 65],
                            ev[:, :nh * 65])

            def emit_kt_pair(w_dram, pair, xT):
                """K^T for one head pair from transposed activations."""
                kt = ktpp.tile([P, TH], f32r, tag="ktp")
                emit_pair_proj(w_dram, pair,
                               lambda d, c0, cw: xT[:, d * TH + c0:
                                                    d * TH + c0 + cw],
                               TH, kt, 0)
                return kt

            def emit_attention(qT, kt, vsc, half, pair, avst, use_masks):
                """One half-T of attention, both heads of a pair.
                qT: [P, OWN] (rows 0:64 head A, 64:128 head B).
                kt: [P, TH]. avst: dict with rolling 'psum' + 'sbuf' [P,2*OWN]
                partial accumulator ([O^T;denom] per head in column halves)."""
                vtt = vstp.tile([P, KBH * 130], f32r, tag="vst")
                nc.sync.dma_start(
                    vtt[:].rearrange("p (kl c) -> p kl c", c=130),
                    vsc[half * TH:(half + 1) * TH,
                        pair * 130:(pair + 1) * 130]
                    .rearrange("(kl p) c -> p kl c", p=P))
                NG = (KBH + 1) // 2
                for hh in range(2):
                    hb = hh * HD
                    for g in range(NG):
                        kls = [kl for kl in (2 * g, 2 * g + 1) if kl < KBH]
                        off = (cfg.off[half * KBH + kls[0]]
                               if use_masks else 0)
                        sc = scps.tile([P, 2 * OWN], f32, tag="sc")
                        for i, kl in enumerate(kls):
                            kg = half * KBH + kl
                            mss = [s for s in range(NQB)
                                   if use_masks and (s, kg) in cfg.mask_idx]
                            nc.tensor.matmul(
                                sc[:, i * OWN + off:(i + 1) * OWN],
                                r(kt[hb:hb + HD, kl * P:(kl + 1) * P]),
                                r(qT[hb:hb + HD, off:]),
                                start=True, stop=(not mss))
                            for n, s in enumerate(mss):
                                idx = cfg.mask_idx[(s, kg)]
                                nc.tensor.matmul(
                                    sc[:, i * OWN + s * P:
                                       i * OWN + (s + 1) * P],
                                    identb[:],
                                    maskt[:, idx * P:(idx + 1) * P],
                                    start=False, stop=(n == len(mss) - 1))
                        pb = pbp.tile([P, 2 * OWN], f32r, tag="pb")
                        if off:
                            nc.scalar.activation(
                                pb[:].rearrange("p (i t) -> p i t",
                                                i=2)[:, :len(kls), off:],
                                sc[:].rearrange("p (i t) -> p i t",
                                                i=2)[:, :len(kls), off:],
                                AF.Exp, scale=scale)
                        else:
                            nc.scalar.activation(pb[:, :len(kls) * OWN],
                                                 sc[:, :len(kls) * OWN],
                                                 AF.Exp, scale=scale)
                        for i, kl in enumerate(kls):
                            nc.tensor.matmul(
                                avst["psum"][0:65, off:OWN],
                                r(vtt[:, kl * 130 + hh * 65:
                                      kl * 130 + (hh + 1) * 65]),
                                r(pb[:, i * OWN + off:(i + 1) * OWN]),
                                start=(g == 0 and i == 0),
                                stop=(g == NG - 1 and i == len(kls) - 1))
                    o0 = cfg.off[half * KBH] if use_masks else 0
                    dst = avst["sbuf"][0:65, hh * OWN + o0:(hh + 1) * OWN]
                    if half == 0:
                        nc.vector.tensor_copy(dst, avst["psum"][0:65, o0:OWN])
                    else:
                        nc.vector.tensor_add(dst, avst["psum"][0:65, o0:OWN],
                                             dst)
                    if not (half == 1 and hh == 1):
                        avst["psum"] = avps.tile([65, OWN], f32, tag="av",
                                                 name=f"avps_{pair}_{half}_{hh}")

            def emit_normalize(avp_sb, hh):
                """O^T /= denominator row, in place in the sbuf partial."""
                cs = hh * OWN
                rec = evp.tile([P, 512], f32r, tag="ev")
                nc.vector.reciprocal(rec[64:65, :OWN],
                                     avp_sb[64:65, cs:cs + OWN])
                bc = avps.tile([64, OWN], f32, tag="av")
                nc.tensor.matmul(bc[:], r(ones65[64:65, :]),
                                 r(rec[64:65, :OWN]), start=True, stop=True)
                bcs = evp.tile([P, 512], f32, tag="ev")
                nc.scalar.activation(bcs[0:64, :OWN], bc[:], AF.Copy)
                nc.vector.tensor_mul(avp_sb[0:64, cs:cs + OWN],
                                     avp_sb[0:64, cs:cs + OWN],
                                     bcs[0:64, :OWN])

            def emit_oproj_residual(wo_dram, bo_t, avp_list, res_tiles):
                """res += transpose(Wo^T @ O^T + bo)   (residual in place)."""
                for m in range(DC):
                    wot = wbigp.tile([64, 2 * DC * P], f32r, tag="wbig",
                                     name=f"wot{m}")
                    nc.sync.dma_start(
                        wot[:].rearrange("p (a c) -> p a c", c=P),
                        wo_dram[:, m * P:(m + 1) * P]
                        .rearrange("(a p) c -> p a c", p=64))
                    ps = linps.tile([P, 512], f32, tag="lin")
                    for n in range(2 * DC):
                        pair, hh = n // 2, n % 2
                        nc.tensor.matmul(
                            ps[:, :OWN], r(wot[:, n * P:(n + 1) * P]),
                            r(avp_list[pair][0:64, hh * OWN:(hh + 1) * OWN]),
                            start=(n == 0), stop=(n == 2 * DC - 1))
                    ev = evp.tile([P, 512], f32, tag="ev")
                    nc.scalar.activation(ev[:, :OWN], ps[:, :OWN], AF.Identity,
                                         bias=bo_t[:, m:m + 1])
                    ps2 = linps.tile([P, 512], f32, tag="lin")
                    for s in range(NQB):
                        nc.tensor.transpose(ps2[:, s * P:(s + 1) * P],
                                            ev[:, s * P:(s + 1) * P], ident[:])
                    for s in range(NQB):
                        nc.vector.tensor_add(
                            res_tiles[s][:, m * P:(m + 1) * P],
                            ps2[:, s * P:(s + 1) * P],
                            res_tiles[s][:, m * P:(m + 1) * P])

            def emit_lnq(res_tiles_or_dram, gbkey, from_dram):
                """LN own tokens + transpose -> [P, DC*OWN] mega-tile."""
                lnq = lnqp.tile([P, DC * OWN], f32r, tag="lnq")
                lnqv = lnq[:].rearrange("p (d t) -> p d t", t=OWN)
                for s in range(NQB):
                    if from_dram:
                        xt = ldp.tile([P, D], f32, tag="ld")
                        nc.sync.dma_start(
                            xt[:], res_tiles_or_dram[s * P:(s + 1) * P, :])
                    else:
                        xt = res_tiles_or_dram[s]
                    lnt = emit_ln(xt[:], gbkey, inplace=from_dram)
                    emit_transposes(lnt, lnqv[:, :, s * P:(s + 1) * P], gbkey)
                return lnq

            # ================= pipeline =================
            res = []
            for s in range(NQB):
                t = residp.tile([P, D], f32, tag="resid")
                nc.sync.dma_start(t[:], x_own[s * P:(s + 1) * P, :])
                res.append(t)

            # own-token LN1 -> Q_sa^T
            lnq1 = emit_lnq(x_own, 1, True)
            qsaT = []
            for pair in range(PAIRS):
                qt = qTp.tile([P, OWN], f32r, tag="qT")
                emit_pair_proj(
                    wq_sa, pair,
                    lambda d, c0, cw: lnq1[:, d * OWN + c0:d * OWN + c0 + cw],
                    OWN, qt, 0)
                qsaT.append(qt)

            # SA attention in two half-T passes
            av_sa = [avpp.tile([P, 2 * OWN], f32r, tag="avp", name=f"av_sa{_pp}")
                     for _pp in range(PAIRS)]
            avst_sa = {}
            for half in range(2):
                xT = xTp.tile([P, DC * TH], f32r, tag="xT")
                ln_transpose_stream(x_dec, half * TH, TH, xT, 1)
                emit_v_to_scratch(wv_sa, xT, half, vsc_sa)
                for pair in range(PAIRS):
                    kt = emit_kt_pair(wk_sa, pair, xT)
                    if half == 0:
                        avst_sa[pair] = {
                            "psum": avps.tile([65, OWN], f32, tag="av",
                                              name=f"avps_sa{pair}"),
                            "sbuf": av_sa[pair]}
                    emit_attention(qsaT[pair], kt, vsc_sa, half, pair,
                                   avst_sa[pair], True)
                    if half == 1:
                        emit_normalize(av_sa[pair], 0)
                        emit_normalize(av_sa[pair], 1)
            emit_oproj_residual(wo_sa, bo_sa_t, av_sa, res)     # res -> x2

            # LN2 -> Q_ca^T
            lnq2 = emit_lnq(res, 2, False)
            qcaT = []
            for pair in range(PAIRS):
                qt = qTp.tile([P, OWN], f32r, tag="qT")
                emit_pair_proj(
                    wq_ca, pair,
                    lambda d, c0, cw: lnq2[:, d * OWN + c0:d * OWN + c0 + cw],
                    OWN, qt, 0)
                qcaT.append(qt)

            # CA attention (raw encoder K/V, no masks)
            av_ca = [avpp.tile([P, 2 * OWN], f32r, tag="avp", name=f"av_ca{_pp}")
                     for _pp in range(PAIRS)]
            avst_ca = {}
            for half in range(2):
                xT = xTp.tile([P, DC * TH], f32r, tag="xT")
                ln_transpose_stream(x_enc, half * TH, TH, xT, None)
                emit_v_to_scratch(wv_ca, xT, half, vsc_ca)
                for pair in range(PAIRS):
                    kt = emit_kt_pair(wk_ca, pair, xT)
                    if half == 0:
                        avst_ca[pair] = {
                            "psum": avps.tile([65, OWN], f32, tag="av",
                                              name=f"avps_ca{pair}"),
                            "sbuf": av_ca[pair]}
                    emit_attention(qcaT[pair], kt, vsc_ca, half, pair,
                                   avst_ca[pair], False)
                    if half == 1:
                        emit_normalize(av_ca[pair], 0)
                        emit_normalize(av_ca[pair], 1)
            emit_oproj_residual(wo_ca, bo_ca_t, av_ca, res)     # res -> x3

            # LN3 -> FFN
            lnq3 = emit_lnq(res, 3, False)
            y2T = [qTp.tile([P, OWN], f32, tag="qT", name=f"y2T{_m}") for _m in range(DC)]
            FG = FC // 2
            for fg in range(2):
                rT = xTp.tile([P, DC * TH], f32r, tag="xT")
                for fi in range(FG):
                    f = fg * FG + fi
                    w1t = wpairp.tile([P, DC * P], f32r, tag="wpair",
                                      name=f"w1t{f}")
                    nc.sync.dma_start(
                        w1t[:].rearrange("p (d c) -> p d c", c=P),
                        w1[:, f * P:(f + 1) * P]
                        .rearrange("(d p) c -> p d c", p=P))
                    ps = linps.tile([P, 512], f32, tag="lin")
                    for d in range(DC):
                        nc.tensor.matmul(
                            ps[:, :OWN], r(w1t[:, d * P:(d + 1) * P]),
                            r(lnq3[:, d * OWN:(d + 1) * OWN]),
                            start=(d == 0), stop=(d == DC - 1))
                    nc.scalar.activation(rT[:, fi * OWN:(fi + 1) * OWN],
                                         ps[:, :OWN], AF.Relu,
                                         bias=b1_t[:, f:f + 1])
                for m in range(DC):
                    w2t = wbigp.tile([P, FG * P], f32r, tag="wbig",
                                     name=f"w2t{fg}{m}")
                    nc.sync.dma_start(
                        w2t[:].rearrange("p (a c) -> p a c", c=P),
                        w2[fg * FG * P:(fg + 1) * FG * P, m * P:(m + 1) * P]
                        .rearrange("(a p) c -> p a c", p=P))
                    ps = linps.tile([P, 512], f32, tag="lin")
                    for fi in range(FG):
                        nc.tensor.matmul(
                            ps[:, :OWN], r(w2t[:, fi * P:(fi + 1) * P]),
                            r(rT[:, fi * OWN:(fi + 1) * OWN]),
                            start=(fi == 0), stop=(fi == FG - 1))
                    if fg == 0:
                        nc.scalar.activation(y2T[m][:], ps[:, :OWN],
                                             AF.Identity,
                                             bias=b2_t[:, m:m + 1])
                    else:
                        nc.vector.tensor_add(y2T[m][:], ps[:, :OWN],
                                             y2T[m][:])

            for m in range(DC):
                ps2 = linps.tile([P, 512], f32, tag="lin")
                for s in range(NQB):
                    nc.tensor.transpose(ps2[:, s * P:(s + 1) * P],
                                        y2T[m][:, s * P:(s + 1) * P], ident[:])
                for s in range(NQB):
                    nc.vector.tensor_add(res[s][:, m * P:(m + 1) * P],
                                         ps2[:, s * P:(s + 1) * P],
                                         res[s][:, m * P:(m + 1) * P])
            for s in range(NQB):
                nc.sync.dma_start(out[s * P:(s + 1) * P, :], res[s][:])

    nc.compile()
    return nc


def own_token_rows(cfg, j):
    return np.concatenate(
        [np.arange(P * (cfg.NQB * s + j), P * (cfg.NQB * s + j) + P)
         for s in range(cfg.NQB)])


def prep_core_inputs(cfg, inputs, core):
    """Host-side slicing/packing for one core."""
    D, H = cfg.D, cfg.H
    b, j = core // 4, core % 4
    a = lambda x: np.asarray(x)
    f32c = lambda x: np.ascontiguousarray(a(x), dtype=np.float32)
    pack_pairs = lambda w: np.ascontiguousarray(np.stack(
        [np.concatenate([a(w)[2 * p], a(w)[2 * p + 1]], axis=1)
         for p in range(cfg.PAIRS)]), dtype=np.float32)
    vall = lambda w: np.ascontiguousarray(
        a(w).transpose(1, 0, 2).reshape(D, H * HD), dtype=np.float32)

    rows = own_token_rows(cfg, j)
    return {
        "x_dec": f32c(a(inputs["decoder_x"])[b]),
        "x_enc": f32c(a(inputs["encoder_x"])[b]),
        "x_own": f32c(a(inputs["decoder_x"])[b][rows]),
        "wq_sa": pack_pairs(inputs["Wq_sa"]),
        "wk_sa": pack_pairs(inputs["Wk_sa"]),
        "wv_sa": vall(inputs["Wv_sa"]),
        "wo_sa": f32c(inputs["Wo_sa"]),
        "bo_sa": f32c(inputs["bo_sa"]),
        "wq_ca": pack_pairs(inputs["Wq_ca"]),
        "wk_ca": pack_pairs(inputs["Wk_ca"]),
        "wv_ca": vall(inputs["Wv_ca"]),
        "wo_ca": f32c(inputs["Wo_ca"]),
        "bo_ca": f32c(inputs["bo_ca"]),
        "w1": f32c(inputs["W1"]),
        "b1": f32c(inputs["b1"]),
        "w2": f32c(inputs["W2"]),
        "b2": f32c(inputs["b2"]),
        "masks": build_masks(cfg, j).astype(ml_dtypes.bfloat16),
    }, rows


def gb_trivial(inputs):
    return all(np.allclose(np.asarray(inputs[g]), 1.0)
               for g in ("g1", "g2", "g3")) and \
           all(np.allclose(np.asarray(inputs[b]), 0.0)
               for b in ("be1", "be2", "be3"))


def run(inputs, trace=False, **rk):
    """Build + run on 8 cores; returns (full_output, BassKernelResults)."""
    from concourse.bass_utils import run_bass_kernel_spmd

    cfg = Cfg()
    with_gb = not gb_trivial(inputs)
    nc = build_nc(cfg, with_gb)

    in_maps, rows_all = [], []
    for core in range(8):
        im, rows = prep_core_inputs(cfg, inputs, core)
        if with_gb:
            for n in ("g1", "be1", "g2", "be2", "g3", "be3"):
                im[n] = np.ascontiguousarray(np.asarray(inputs[n]),
                                             dtype=np.float32)
        in_maps.append(im)
        rows_all.append(rows)

    res = run_bass_kernel_spmd(nc, in_maps, list(range(8)), trace=trace, **rk)
    full = np.zeros((2, cfg.T, cfg.D), np.float32)
    for core in range(8):
        full[core // 4][rows_all[core]] = res.results[core]["out"]
    return full, res


def kernel(**inputs) -> np.ndarray:
    return run(inputs)[0]

